# revision 19
# baseline (speedup 1.0000x reference)
"""BertAttention (QKV proj + MHA + out-proj + residual + LayerNorm) on 8
Trainium2 NeuronCores.

Sharding: tensor-parallel over heads. Core c owns heads {2c, 2c+1} (a
128-wide slice of the hidden dim): it computes Q/K/V projections for its
slice over the full batch*seq, runs attention for its 8 (batch, head)
pairs entirely out of SBUF, then an AllToAll re-shards the attention
context from head-split to sequence-split so each core runs the output
projection + residual + LayerNorm on its 1024-row shard of the flattened
(B*S) dimension. Host concatenates the 8 row-shards.

Matmuls run in bf16 (fp32 PSUM accumulate); softmax and LayerNorm
arithmetic stay fp32. The attention-path error this introduces is
suppressed ~100x in the final output by the fp32 residual.

softmax: scores are built transposed (scoresT[k, q] per head) so the
probs@V contraction needs no transpose; the row-sum comes from an extra
all-ones column appended to V; the attention mask enters as
exp(mask[b,k]) folded into V's rows and the ones column (exp(s+m) =
exp(s)*exp(m), and mask only depends on the key position).
"""

import os
import sys
import contextlib
import ctypes
import types

import numpy as np
import ml_dtypes

N_CORES = 8
B, S, H = 4, 2048, 1024
NH, DH = 16, 64
R = B * S            # 8192 flattened rows
RS = R // N_CORES    # 1024 rows per core (output shard)
HB = H // N_CORES    # 128 head-dim columns per core (2 heads)
SBW = 512            # seq-block width for projections
NSB = R // SBW       # 16 seq blocks
NHC = H // 128       # 8 contraction chunks of 128
NKB = S // 128       # 16 key blocks per batch
LN_EPS = 1e-12

last_exec_time_ns = None

# ---------------------------------------------------------------------------
# NTFF profile hook shim (axon images without antenv.axon_hooks).
# Only needed when tracing; harmless otherwise.
_SO_PATH = "/opt/axon/libaxon_pjrt.so"


def _install_ntff_shim():
    try:
        from antenv import axon_hooks  # noqa: F401
        return
    except ImportError:
        pass
    hook = None
    try:
        lib = ctypes.CDLL(_SO_PATH)
        if hasattr(lib, "axon_start_nrt_profile"):
            lib.axon_start_nrt_profile.argtypes = [
                ctypes.POINTER(ctypes.c_int64), ctypes.c_size_t]
            lib.axon_start_nrt_profile.restype = ctypes.c_int64
            lib.axon_stop_nrt_profile.argtypes = [ctypes.c_char_p]
            lib.axon_stop_nrt_profile.restype = ctypes.c_int64

            @contextlib.contextmanager
            def _hook(output_dir, device_ids):
                import jax
                jax.devices()
                if device_ids:
                    ids = (ctypes.c_int64 * len(device_ids))(*device_ids)
                    rc = lib.axon_start_nrt_profile(ids, len(device_ids))
                else:
                    rc = lib.axon_start_nrt_profile(None, 0)
                if rc != 0:
                    raise RuntimeError(f"axon_start_nrt_profile rc={rc}")
                try:
                    yield
                finally:
                    n = lib.axon_stop_nrt_profile(str(output_dir).encode())
                    print(f"profile: {n} ntff file(s) in {output_dir}",
                          file=sys.stderr)

            hook = _hook
    except OSError:
        pass
    mod = types.ModuleType("antenv.axon_hooks")
    mod._hook = hook
    mod.get_axon_ntff_profile_hook = lambda: mod._hook
    mod.set_axon_ntff_profile_hook = lambda h: setattr(mod, "_hook", h)
    sys.modules["antenv.axon_hooks"] = mod
    try:
        import antenv
        antenv.axon_hooks = mod
    except ImportError:
        pass


# ---------------------------------------------------------------------------

def _build(dbg=False):
    from concourse import bacc, tile
    import concourse.mybir as mybir

    f32 = mybir.dt.float32
    bf16 = mybir.dt.bfloat16
    AF = mybir.ActivationFunctionType
    ALU = mybir.AluOpType

    nc = bacc.Bacc("TRN2", target_bir_lowering=False, debug=False,
                   num_devices=N_CORES)

    # ---- DRAM I/O ----
    hT_d = nc.dram_tensor("hT", [H, R], bf16, kind="ExternalInput")
    wqT_d = nc.dram_tensor("wqT", [H, HB], bf16, kind="ExternalInput")
    wkT_d = nc.dram_tensor("wkT", [H, HB], bf16, kind="ExternalInput")
    wvT_d = nc.dram_tensor("wvT", [H, HB], bf16, kind="ExternalInput")
    woT_d = nc.dram_tensor("woT", [H, H], bf16, kind="ExternalInput")
    bq_d = nc.dram_tensor("bq", [HB], f32, kind="ExternalInput")
    bk_d = nc.dram_tensor("bk", [HB], f32, kind="ExternalInput")
    bv_d = nc.dram_tensor("bv", [HB], f32, kind="ExternalInput")
    bo_d = nc.dram_tensor("bo", [H], f32, kind="ExternalInput")
    gamma_d = nc.dram_tensor("gamma", [H], f32, kind="ExternalInput")
    beta_d = nc.dram_tensor("beta", [H], f32, kind="ExternalInput")
    maskT_d = nc.dram_tensor("maskT", [S, B], f32, kind="ExternalInput")
    hres_d = nc.dram_tensor("hres", [RS, H], f32, kind="ExternalInput")
    out_d = nc.dram_tensor("out", [RS, H], f32, kind="ExternalOutput")
    if dbg:
        dbg_qt = nc.dram_tensor("dbg_qt", [128, NSB, SBW], f32,
                                kind="ExternalOutput")
        dbg_kt = nc.dram_tensor("dbg_kt", [128, NSB, SBW], f32,
                                kind="ExternalOutput")
        dbg_va = nc.dram_tensor("dbg_va", [2, 128, NKB, DH + 1], f32,
                                kind="ExternalOutput")
        dbg_ctxT = nc.dram_tensor("dbg_ctxT", [128, NSB, SBW], f32,
                                  kind="ExternalOutput")
        dbg_ctxF = nc.dram_tensor("dbg_ctxF", [128, NHC, RS], f32,
                                  kind="ExternalOutput")

    with tile.TileContext(nc) as tc:
        with (
            tc.tile_pool(name="const", bufs=1) as cpool,
            tc.tile_pool(name="psA", bufs=2, space="PSUM") as psA,
            tc.tile_pool(name="psB", bufs=2, space="PSUM") as psB,
            tc.tile_pool(name="psC", bufs=2, space="PSUM") as psC,
            tc.tile_pool(name="dram", bufs=1, space="DRAM") as dpool,
        ):
            # ================= setup =================
            wq_sb = cpool.tile([128, NHC, HB], bf16, tag="wq")
            wk_sb = cpool.tile([128, NHC, HB], bf16, tag="wk")
            wv_sb = cpool.tile([128, NHC, HB], bf16, tag="wv")
            for c in range(NHC):
                nc.sync.dma_start(wq_sb[:, c, :], wqT_d[128 * c:128 * (c + 1), :])
                nc.sync.dma_start(wk_sb[:, c, :], wkT_d[128 * c:128 * (c + 1), :])
                nc.sync.dma_start(wv_sb[:, c, :], wvT_d[128 * c:128 * (c + 1), :])

            bq_sb = cpool.tile([128, 1], f32, tag="bq")
            bk_sb = cpool.tile([128, 1], f32, tag="bk")
            nc.sync.dma_start(bq_sb[:, :], bq_d[:].unsqueeze(1))
            nc.sync.dma_start(bk_sb[:, :], bk_d[:].unsqueeze(1))

            # bv broadcast along partitions (V is in [seq, d] layout)
            bv_b = cpool.tile([128, HB], f32, tag="bv_b")
            nc.sync.dma_start(bv_b[:, :],
                              bv_d[:].unsqueeze(0).partition_broadcast(128))

            # exp(mask), laid out [k-in-block, kblock, batch]
            em_sb = cpool.tile([128, NKB, B], f32, tag="em")
            for kb in range(NKB):
                nc.sync.dma_start(em_sb[:, kb, :],
                                  maskT_d[128 * kb:128 * (kb + 1), :])
            nc.scalar.activation(em_sb[:, :, :], em_sb[:, :, :], AF.Exp)

            # attention context, persistent until the AllToAll
            ctxT_sb = cpool.tile([128, NSB, SBW], bf16, tag="ctxT")

            # ============ phases 1+2: projections + attention =============
            with (
                tc.tile_pool(name="attn", bufs=2) as apool,
                tc.tile_pool(name="ptp", bufs=3) as ptpool,
            ):
                qt_sb = apool.tile([128, NSB, SBW], bf16, tag="qt", bufs=1)
                kt_sb = apool.tile([128, NSB, SBW], bf16, tag="kt", bufs=1)
                for b in range(B):
                    va = [apool.tile([128, NKB, DH + 1], bf16, tag=f"va{h}",
                                     name=f"va{h}")
                          for h in range(2)]
                    for i in range(4 * b, 4 * b + 4):
                        # hidden^T block [H, SBW] -> [128, NHC, SBW]
                        hsb = apool.tile([128, NHC, SBW], bf16, tag="hsb")
                        for c in range(NHC):
                            nc.sync.dma_start(
                                hsb[:, c, :],
                                hT_d[128 * c:128 * (c + 1),
                                     SBW * i:SBW * (i + 1)])
                        # Q^T block
                        pq = psA.tile([128, SBW], f32, tag="proj")
                        for c in range(NHC):
                            nc.tensor.matmul(pq[:, :], wq_sb[:, c, :],
                                             hsb[:, c, :],
                                             start=(c == 0),
                                             stop=(c == NHC - 1))
                        nc.vector.tensor_scalar_add(qt_sb[:, i, :], pq[:, :],
                                                    bq_sb[:, :])
                        # K^T block
                        pk = psA.tile([128, SBW], f32, tag="proj")
                        for c in range(NHC):
                            nc.tensor.matmul(pk[:, :], wk_sb[:, c, :],
                                             hsb[:, c, :],
                                             start=(c == 0),
                                             stop=(c == NHC - 1))
                        nc.vector.tensor_scalar_add(kt_sb[:, i, :], pk[:, :],
                                                    bk_sb[:, :])
                        # V in natural [seq, d] layout, 4 sub-blocks of 128
                        for sub in range(4):
                            kb = 4 * (i - 4 * b) + sub  # key block in batch
                            pv = psA.tile([128, SBW], f32, tag="proj")
                            for c in range(NHC):
                                nc.tensor.matmul(
                                    pv[:, 0:HB],
                                    hsb[:, c, 128 * sub:128 * (sub + 1)],
                                    wv_sb[:, c, :],
                                    start=(c == 0), stop=(c == NHC - 1))
                            emcol = em_sb[:, kb, b].unsqueeze(1)
                            t1 = apool.tile([128, HB], f32, tag="t1")
                            nc.vector.tensor_add(t1[:, :], pv[:, 0:HB],
                                                 bv_b[:, :])
                            for h in range(2):
                                nc.vector.tensor_scalar_mul(
                                    va[h][:, kb, 0:DH],
                                    t1[:, DH * h:DH * (h + 1)], emcol)
                                nc.vector.tensor_copy(va[h][:, kb, DH:DH + 1],
                                                      emcol)

                    # ---- attention for batch b ----
                    for qg in range(4):
                        blk = 4 * b + qg
                        pc0 = psC.tile([DH + 1, SBW], f32, tag="ctx")
                        pc1 = psC.tile([DH + 1, SBW], f32, tag="ctx")
                        for kb in range(NKB):
                            sblk = 4 * b + kb // 4
                            kcol = 128 * (kb % 4)
                            sc = psB.tile([128, 2 * SBW], f32, tag="sc")
                            nc.tensor.matmul(
                                sc[:, 0:SBW],
                                kt_sb[0:DH, sblk, kcol:kcol + 128],
                                qt_sb[0:DH, blk, :],
                                start=True, stop=True)
                            nc.tensor.matmul(
                                sc[:, SBW:2 * SBW],
                                kt_sb[DH:2 * DH, sblk, kcol:kcol + 128],
                                qt_sb[DH:2 * DH, blk, :],
                                start=True, stop=True)
                            pt = ptpool.tile([128, 2 * SBW], bf16, tag="pt")
                            nc.scalar.activation(pt[:, :], sc[:, :], AF.Exp,
                                                 scale=0.125)
                            nc.tensor.matmul(pc0[:, :], va[0][:, kb, :],
                                             pt[:, 0:SBW],
                                             start=(kb == 0),
                                             stop=(kb == NKB - 1))
                            nc.tensor.matmul(pc1[:, :], va[1][:, kb, :],
                                             pt[:, SBW:2 * SBW],
                                             start=(kb == 0),
                                             stop=(kb == NKB - 1))
                        # normalize: ctxT[d, q] = ctx'[d, q] / rowsum[q]
                        # partition_broadcast is only correct with base
                        # partition 0 on both sides -> base-0 tiles per head.
                        rs = apool.tile([1, 2 * SBW], f32, tag="rs")
                        nc.vector.tensor_copy(rs[:, 0:SBW], pc0[DH:DH + 1, :])
                        nc.vector.tensor_copy(rs[:, SBW:2 * SBW],
                                              pc1[DH:DH + 1, :])
                        # transpose into partitions for lane-parallel
                        # reciprocal; shape-mismatched DMAs linearize both
                        # sides in AP order, so fwd and back use the same
                        # bijection.
                        rsT = apool.tile([128, 8], f32, tag="rsT")
                        for h in range(2):
                            nc.sync.dma_start(rsT[:, 4 * h:4 * (h + 1)],
                                              rs[:, SBW * h:SBW * (h + 1)])
                        rcT = apool.tile([128, 8], f32, tag="rcT")
                        nc.vector.reciprocal(rcT[:, :], rsT[:, :])
                        rc = apool.tile([1, 2 * SBW], f32, tag="rc")
                        for h in range(2):
                            nc.sync.dma_start(rc[:, SBW * h:SBW * (h + 1)],
                                              rcT[:, 4 * h:4 * (h + 1)])
                        rb = [apool.tile([DH, SBW], f32, tag=f"rb{h}",
                                         name=f"rb{h}") for h in range(2)]
                        for h in range(2):
                            nc.gpsimd.partition_broadcast(
                                rb[h][:, :], rc[:, SBW * h:SBW * (h + 1)])
                        nc.vector.tensor_mul(ctxT_sb[0:DH, blk, :],
                                             pc0[0:DH, :], rb[0][:, :])
                        nc.vector.tensor_mul(ctxT_sb[DH:2 * DH, blk, :],
                                             pc1[0:DH, :], rb[1][:, :])
                    if dbg and b == 0:
                        for h in range(2):
                            nc.gpsimd.dma_start(dbg_va[h, :, :, :],
                                                va[h][:, :, :])

                if dbg:
                    nc.gpsimd.dma_start(dbg_qt[:, :, :], qt_sb[:, :, :])
                    nc.gpsimd.dma_start(dbg_kt[:, :, :], kt_sb[:, :, :])
                    nc.gpsimd.dma_start(dbg_ctxT[:, :, :], ctxT_sb[:, :, :])

            # ================= AllToAll: head-split -> seq-split ==========
            a2a_in = dpool.tile([N_CORES, 128, RS], bf16, tag="a2a_in")
            a2a_out = dpool.tile([N_CORES, 128, RS], bf16, tag="a2a_out")
            for j in range(N_CORES):
                nc.sync.dma_start(a2a_in[j, :, :],
                                  ctxT_sb[:, 2 * j:2 * j + 2, :])
            nc.gpsimd.collective_compute(
                "AllToAll", ALU.bypass,
                replica_groups=[list(range(N_CORES))],
                ins=[a2a_in[:].opt()], outs=[a2a_out[:].opt()])

            # ============ phases 3+4: out-proj + residual + LayerNorm =====
            with tc.tile_pool(name="outp", bufs=2) as opool:
                ctxF_sb = opool.tile([128, NHC, RS], bf16, tag="ctxF", bufs=1)
                for src in range(N_CORES):
                    nc.sync.dma_start(ctxF_sb[:, src, :], a2a_out[src, :, :])
                if dbg:
                    nc.gpsimd.dma_start(dbg_ctxF[:, :, :], ctxF_sb[:, :, :])

                wo_sb = opool.tile([128, NHC, H], bf16, tag="wo", bufs=1)
                for c in range(NHC):
                    nc.sync.dma_start(wo_sb[:, c, :],
                                      woT_d[128 * c:128 * (c + 1), :])
                hres_sb = opool.tile([128, RS // 128, H], f32, tag="hres",
                                     bufs=1)
                for t in range(RS // 128):
                    nc.sync.dma_start(hres_sb[:, t, :],
                                      hres_d[128 * t:128 * (t + 1), :])
                # broadcast bo/gamma/beta along partitions; fold bo into hres
                bo_b = opool.tile([128, H], f32, tag="bo_b", bufs=1)
                gamma_b = opool.tile([128, H], f32, tag="gamma_b", bufs=1)
                beta_b = opool.tile([128, H], f32, tag="beta_b", bufs=1)
                nc.sync.dma_start(
                    bo_b[:, :], bo_d[:].unsqueeze(0).partition_broadcast(128))
                nc.sync.dma_start(
                    gamma_b[:, :],
                    gamma_d[:].unsqueeze(0).partition_broadcast(128))
                nc.sync.dma_start(
                    beta_b[:, :],
                    beta_d[:].unsqueeze(0).partition_broadcast(128))
                for t in range(RS // 128):
                    nc.vector.tensor_add(hres_sb[:, t, :], hres_sb[:, t, :],
                                         bo_b[:, :])

                inv_h = float(1.0 / H)
                for t in range(RS // 128):
                    x_sb = opool.tile([128, H], f32, tag="xln")
                    for g in range(2):
                        po = psA.tile([128, SBW], f32, tag="proj")
                        for c in range(NHC):
                            nc.tensor.matmul(
                                po[:, :],
                                ctxF_sb[:, c, 128 * t:128 * (t + 1)],
                                wo_sb[:, c, SBW * g:SBW * (g + 1)],
                                start=(c == 0), stop=(c == NHC - 1))
                        nc.vector.tensor_add(
                            x_sb[:, SBW * g:SBW * (g + 1)], po[:, :],
                            hres_sb[:, t, SBW * g:SBW * (g + 1)])
                    ssum = opool.tile([128, 1], f32, tag="ssum")
                    nc.vector.tensor_reduce(ssum[:, :], x_sb[:, :],
                                            mybir.AxisListType.X, ALU.add)
                    negmu = opool.tile([128, 1], f32, tag="negmu")
                    nc.vector.tensor_scalar_mul(negmu[:, :], ssum[:, :],
                                                -inv_h)
                    xc = opool.tile([128, H], f32, tag="xc")
                    nc.vector.tensor_scalar_add(xc[:, :], x_sb[:, :],
                                                negmu[:, :])
                    ssq = opool.tile([128, 1], f32, tag="ssq")
                    # x_sb is dead after centering; reuse as Square scratch
                    nc.scalar.activation(x_sb[:, :], xc[:, :], AF.Square,
                                         accum_out=ssq[:, :])
                    var = opool.tile([128, 1], f32, tag="var")
                    nc.vector.tensor_scalar(var[:, :], ssq[:, :], inv_h,
                                            LN_EPS, ALU.mult, ALU.add)
                    rv = opool.tile([128, 1], f32, tag="rv")
                    nc.vector.reciprocal(rv[:, :], var[:, :])
                    rstd = opool.tile([128, 1], f32, tag="rstd")
                    nc.scalar.activation(rstd[:, :], rv[:, :], AF.Sqrt)
                    y_sb = opool.tile([128, H], f32, tag="yln")
                    nc.vector.scalar_tensor_tensor(y_sb[:, :], xc[:, :],
                                                   rstd[:, :], gamma_b[:, :],
                                                   ALU.mult, ALU.mult)
                    nc.vector.tensor_add(y_sb[:, :], y_sb[:, :], beta_b[:, :])
                    nc.sync.dma_start(out_d[128 * t:128 * (t + 1), :],
                                      y_sb[:, :])

    nc.compile()
    return nc


_NC_CACHE = None


def _get_nc():
    global _NC_CACHE
    if _NC_CACHE is None:
        _NC_CACHE = _build()
    return _NC_CACHE


def _make_in_maps(hidden_states, attention_mask, Wq, bq, Wk, bk, Wv, bv, Wo,
                  bo, ln_gamma, ln_beta):
    hid2 = np.asarray(hidden_states, np.float32).reshape(R, H)
    hT_bf = np.ascontiguousarray(hid2.T).astype(ml_dtypes.bfloat16)
    woT = np.ascontiguousarray(np.asarray(Wo, np.float32).T).astype(
        ml_dtypes.bfloat16)
    maskT = np.ascontiguousarray(
        np.asarray(attention_mask, np.float32).reshape(B, S).T)
    bo32 = np.asarray(bo, np.float32)
    gamma32 = np.asarray(ln_gamma, np.float32)
    beta32 = np.asarray(ln_beta, np.float32)

    in_maps = []
    for c in range(N_CORES):
        sl = slice(HB * c, HB * (c + 1))
        in_maps.append({
            "hT": hT_bf,
            "wqT": np.ascontiguousarray(np.asarray(Wq, np.float32)[sl, :].T
                                        ).astype(ml_dtypes.bfloat16),
            "wkT": np.ascontiguousarray(np.asarray(Wk, np.float32)[sl, :].T
                                        ).astype(ml_dtypes.bfloat16),
            "wvT": np.ascontiguousarray(np.asarray(Wv, np.float32)[sl, :].T
                                        ).astype(ml_dtypes.bfloat16),
            "woT": woT,
            "bq": np.ascontiguousarray(np.asarray(bq, np.float32)[sl]),
            "bk": np.ascontiguousarray(np.asarray(bk, np.float32)[sl]),
            "bv": np.ascontiguousarray(np.asarray(bv, np.float32)[sl]),
            "bo": bo32,
            "gamma": gamma32,
            "beta": beta32,
            "maskT": maskT,
            "hres": np.ascontiguousarray(hid2[RS * c:RS * (c + 1), :]),
        })
    return in_maps


def kernel(hidden_states, attention_mask, Wq, bq, Wk, bk, Wv, bv, Wo, bo,
           ln_gamma, ln_beta):
    global last_exec_time_ns
    from concourse.bass_utils import run_bass_kernel_spmd

    _install_ntff_shim()
    in_maps = _make_in_maps(hidden_states, attention_mask, Wq, bq, Wk, bk,
                            Wv, bv, Wo, bo, ln_gamma, ln_beta)
    nc = _get_nc()
    trace = os.environ.get("BASS_KERNEL_TRACE", "0") == "1"
    res = run_bass_kernel_spmd(nc, in_maps, core_ids=list(range(N_CORES)),
                               trace=trace)
    last_exec_time_ns = res.exec_time_ns
    if trace and res.exec_time_ns is not None:
        print(f"HW exec time: {res.exec_time_ns} ns")

    out = np.concatenate([res.results[c]["out"] for c in range(N_CORES)],
                         axis=0)
    return out.reshape(B, S, H).astype(np.float32)
